# revision 9
# baseline (speedup 1.0000x reference)
"""BSGRU (block-switching GRU) Trainium2 Bass kernel.

Math reformulation (validated vs reference to ~1e-6):
  The torch-style .view() on (ky*w_ih) / (kk*w_hh) is equivalent to dense
  matmuls with per-row block scaling:
    i_gates[b,g] = ky[b, g//192] * (x_t @ W2[g]) + b_ih[g],   W2 = w_ih.reshape(768, in)
    h_gates[b,g] = kx[b, g//192]*ky[b, (g%192)//48] * (h @ U2[g]) + b_hh[g]
  sigma(a) = (1+tanh(a/2))/2 so only Exp+Tanh are needed (one ACT table set).

Layout: batch (8) on partitions, gates (768) on free dim.  Per-layer:
  bulk precompute  ibase = X @ W2^T (+ route-x logits) on PE, then a
  64-step scan; state h kept transposed (two (128,8) column blocks per step)
  via per-step PE transposes so the next step's matmuls can contract over h.
All 8 cores run the identical program (scan is latency-bound; extra cores
cannot shorten the sequential critical path) and core 0's output is used.
"""

import functools
import numpy as np

S, B, H, K, BS, I_SZ = 64, 8, 256, 4, 64, 256
G3 = 3 * H  # 768
BETA = 10.0
NSTEP_DEFAULT = S


@functools.lru_cache(maxsize=2)
def _build(n_steps: int = NSTEP_DEFAULT, use_f32r: bool = True):
    import concourse.bass as bass
    import concourse.bacc as bacc
    import concourse.mybir as mybir
    import concourse.tile as tile

    f32 = mybir.dt.float32
    f32r = mybir.dt.float32r
    AF = mybir.ActivationFunctionType
    ALU = mybir.AluOpType

    mmdt = f32r if use_f32r else f32

    nc = bacc.Bacc("TRN2", target_bir_lowering=False, debug=False)

    def din(name, shape, dt=f32):
        return nc.dram_tensor(name, list(shape), dt, kind="ExternalInput").ap()

    ST = n_steps
    xT_d = din("xT", (2, 128, 8 * ST), mmdt)        # x^T h-chunked
    wbig_d = din("wbig", (2, 2, 128, 772), mmdt)    # [W2^T | beta*w_ik] per layer/chunk
    u2t_d = din("u2t", (2, 2, 128, 768), mmdt)
    whkb_d = din("whkb", (2, 2, 128, 4), mmdt)      # beta*w_hk
    rbig_d = din("rbig", (2, 128, 772))       # [0..0 | beta*b_ik] row-replicated
    bbhzr_d = din("bbhzr", (2, 8, 512))       # (b_ih+b_hh)[:512]/2
    bihn_d = din("bihn", (2, 8, 256))         # b_ih[512:]
    bhhn2_d = din("bhhn2", (2, 8, 256))       # b_hh[512:]/2
    kyinit_d = din("kyinit", (8, 4))
    zeros_d = din("zeros8", (128, 8), mmdt)
    ident_d = din("ident", (8, 8))

    hT_d = nc.dram_tensor("hT_out", [2, 2, 128, 8 * ST], mmdt, kind="ExternalOutput").ap()
    blk_d = nc.dram_tensor("blk_out", [2, ST, B, K], f32, kind="ExternalOutput").ap()

    def sb(name, shape, dt=f32):
        return nc.alloc_sbuf_tensor(name, list(shape), dt).ap()

    # persistent SBUF
    xT_s = [sb(f"xT{c}", (128, 8 * ST), mmdt) for c in range(2)]
    wbig_s = [[sb(f"wbig{l}{c}", (128, 772), mmdt) for c in range(2)] for l in range(2)]
    u2t_s = [[sb(f"u2t{l}{c}", (128, 768), mmdt) for c in range(2)] for l in range(2)]
    whkb_s = [[sb(f"whkb{l}{c}", (128, 4), mmdt) for c in range(2)] for l in range(2)]
    rbig_s = [sb(f"rbig{l}", (128, 772)) for l in range(2)]
    bbhzr_s = [sb(f"bbhzr{l}", (8, 512)) for l in range(2)]
    bihn_s = [sb(f"bihn{l}", (8, 256)) for l in range(2)]
    bhhn2_s = [sb(f"bhhn2{l}", (8, 256)) for l in range(2)]
    ident_s = sb("ident_sb", (8, 8))
    bulk_s = [sb(f"bulk{m}", (128, 772)) for m in range((8 * ST + 127) // 128)]
    hT_s = [[sb(f"hT{l}{c}", (128, 8 * (ST + 1)), mmdt) for c in range(2)] for l in range(2)]
    ky_s = [sb(f"kyall{l}", (8, 4 * (ST + 1))) for l in range(2)]
    kyh_s = [sb(f"kyhall{l}", (8, 4 * (ST + 1))) for l in range(2)]
    hy0_s = sb("hy0", (8, 256))

    n_mt = (8 * ST + 127) // 128  # M-tiles in bulk matmul

    with tile.TileContext(nc) as tc:
        with (
            tc.tile_pool(name="ld", bufs=6) as p_ld,
            tc.tile_pool(name="sm", bufs=3) as p_sm,
            tc.tile_pool(name="mid", bufs=2) as p_mid,
            tc.tile_pool(name="hy", bufs=3) as p_hy,
            tc.tile_pool(name="psG", bufs=2, space="PSUM") as p_psG,
            tc.tile_pool(name="psR", bufs=1, space="PSUM") as p_psR,
            tc.tile_pool(name="psT", bufs=1, space="PSUM") as p_psT,
            tc.tile_pool(name="psB", bufs=1, space="PSUM") as p_psB,
        ):
            # ---- init loads ----
            for c in range(2):
                nc.sync.dma_start(xT_s[c], xT_d[c])
            for l in range(2):
                for c in range(2):
                    nc.sync.dma_start(wbig_s[l][c], wbig_d[l, c])
                    nc.sync.dma_start(u2t_s[l][c], u2t_d[l, c])
                    nc.sync.dma_start(whkb_s[l][c], whkb_d[l, c])
                nc.sync.dma_start(rbig_s[l], rbig_d[l])
                nc.sync.dma_start(bbhzr_s[l], bbhzr_d[l])
                nc.sync.dma_start(bihn_s[l], bihn_d[l])
                nc.sync.dma_start(bhhn2_s[l], bhhn2_d[l])
                nc.sync.dma_start(ky_s[l][:, 0:4], kyinit_d)
                nc.scalar.activation(kyh_s[l][:, 0:4], ky_s[l][:, 0:4],
                                     AF.Copy, scale=0.5)
                for c in range(2):
                    nc.sync.dma_start(hT_s[l][c][:, 0:8], zeros_d)
            nc.sync.dma_start(ident_s, ident_d)
            nc.vector.memset(hy0_s, 0.0)

            def emit_bulk(l):
                # ibase/route-x for layer l -> bulk_s tiles (each (128,772))
                for m in range(n_mt):
                    mw = min(128, 8 * ST - 128 * m)
                    psB = p_psB.tile([mw, 772], f32, tag="psB")
                    if l == 0:
                        lhs = [xT_s[c][:, 128 * m:128 * m + mw] for c in range(2)]
                    else:
                        lhs = [hT_s[0][c][:, 8 + 128 * m:8 + 128 * m + mw]
                               for c in range(2)]
                    for lo, hi in ((0, 512), (512, 772)):
                        for c in range(2):
                            nc.tensor.matmul(
                                psB[:, lo:hi], (lhs[c]),
                                (wbig_s[l][c][:, lo:hi]),
                                start=(c == 0), stop=(c == 1))
                    nc.vector.tensor_tensor(bulk_s[m][0:mw, :], psB, rbig_s[l][0:mw, :], ALU.add)

            def emit_step(l, t, prev_hy):
                mt, ro = (8 * t) // 128, (8 * t) % 128
                ibrx = p_ld.tile([8, 772], f32, tag="ibrx")
                nc.sync.dma_start(ibrx, bulk_s[mt][ro:ro + 8, :])

                hT_c = [hT_s[l][c][:, 8 * t:8 * t + 8] for c in range(2)]
                psR = p_psR.tile([8, 4], f32, tag="psR")
                for c in range(2):
                    nc.tensor.matmul(psR, (hT_c[c]), (whkb_s[l][c]),
                                     start=(c == 0), stop=(c == 1))
                psG = p_psG.tile([8, 768], f32, tag="psG")
                for lo, hi in ((0, 512), (512, 768)):
                    for c in range(2):
                        nc.tensor.matmul(psG[:, lo:hi], (hT_c[c]),
                                         (u2t_s[l][c][:, lo:hi]),
                                         start=(c == 0), stop=(c == 1))

                rin = p_sm.tile([8, 4], f32, tag="rin")
                nc.vector.tensor_tensor(rin, psR, ibrx[:, 768:772], ALU.add)
                e_t = p_sm.tile([8, 4], f32, tag="e")
                s_t = p_sm.tile([8, 1], f32, tag="s")
                nc.scalar.activation(e_t, rin, AF.Exp, accum_out=s_t)
                rs = p_sm.tile([8, 1], f32, tag="rs")
                nc.vector.reciprocal(rs, s_t)
                ky_sl = ky_s[l][:, 4 * (t + 1):4 * (t + 1) + 4]
                kx_sl = ky_s[l][:, 4 * t:4 * t + 4]
                nc.vector.tensor_scalar_mul(ky_sl, e_t, rs)
                kyh_sl = kyh_s[l][:, 4 * (t + 1):4 * (t + 1) + 4]
                kxh_sl = kyh_s[l][:, 4 * t:4 * t + 4]
                nc.vector.tensor_scalar_mul(kyh_sl, ky_sl, 0.5)

                kk = p_sm.tile([8, 16], f32, tag="kk")
                kx_r = kx_sl.unsqueeze(2).broadcast_to((8, 4, 4))
                ky_r = ky_sl.unsqueeze(1).broadcast_to((8, 4, 4))
                nc.gpsimd.tensor_tensor(
                    kk.rearrange("p (a b) -> p a b", a=4), kx_r, ky_r, ALU.mult)

                T1 = p_mid.tile([8, 768], f32, tag="T1")
                ky_b = ky_sl.unsqueeze(2).broadcast_to((8, 4, 192))
                nc.vector.tensor_tensor(
                    T1.rearrange("p (a c) -> p a c", a=4), ky_b,
                    ibrx[:, 0:768].rearrange("p (a c) -> p a c", a=4), ALU.mult)
                T2 = p_mid.tile([8, 768], f32, tag="T2")
                kk_b = kk.unsqueeze(2).broadcast_to((8, 16, 48))
                nc.vector.tensor_tensor(
                    T2.rearrange("p (a c) -> p a c", a=16), kk_b,
                    psG.rearrange("p (a c) -> p a c", a=16), ALU.mult)

                Pzr = p_mid.tile([8, 512], f32, tag="Pzr")
                nc.vector.scalar_tensor_tensor(
                    Pzr, T1[:, 0:512], 0.5, bbhzr_s[l], ALU.mult, ALU.add)
                Qzr = p_mid.tile([8, 512], f32, tag="Qzr")
                nc.vector.scalar_tensor_tensor(
                    Qzr, T2[:, 0:512], 0.5, Pzr, ALU.mult, ALU.add)
                TZR = p_mid.tile([8, 512], f32, tag="TZR")
                nc.scalar.activation(TZR[:, 256:512], Qzr[:, 256:512], AF.Tanh)
                nc.scalar.activation(TZR[:, 0:256], Qzr[:, 0:256], AF.Tanh)

                IN = p_mid.tile([8, 256], f32, tag="IN")
                nc.gpsimd.tensor_tensor(IN, T1[:, 512:768], bihn_s[l], ALU.add)
                HN2 = p_mid.tile([8, 256], f32, tag="HN2")
                nc.vector.scalar_tensor_tensor(
                    HN2, T2[:, 512:768], 0.5, bhhn2_s[l], ALU.mult, ALU.add)
                V = p_mid.tile([8, 256], f32, tag="V")
                nc.gpsimd.tensor_tensor(V, HN2, TZR[:, 256:512], ALU.mult)
                W1 = p_mid.tile([8, 256], f32, tag="W1")
                nc.gpsimd.tensor_tensor(W1, IN, HN2, ALU.add)
                narg = p_mid.tile([8, 256], f32, tag="narg")
                nc.gpsimd.tensor_tensor(narg, W1, V, ALU.add)
                TN = p_mid.tile([8, 256], f32, tag="TN")
                nc.scalar.activation(TN, narg, AF.Tanh)

                kxh64 = kxh_sl.unsqueeze(2).broadcast_to((8, 4, 64))
                kyh64 = kyh_sl.unsqueeze(2).broadcast_to((8, 4, 64))
                A2 = p_mid.tile([8, 256], f32, tag="A2")
                nc.gpsimd.tensor_tensor(
                    A2.rearrange("p (a c) -> p a c", a=4),
                    prev_hy.rearrange("p (a c) -> p a c", a=4),
                    kxh64, ALU.mult)
                B2 = p_mid.tile([8, 256], f32, tag="B2")
                nc.gpsimd.tensor_tensor(
                    B2.rearrange("p (a c) -> p a c", a=4),
                    TN.rearrange("p (a c) -> p a c", a=4),
                    kyh64, ALU.mult)
                F1 = p_mid.tile([8, 256], f32, tag="F1")
                nc.vector.scalar_tensor_tensor(
                    F1, TZR[:, 0:256], 1.0, A2, ALU.add, ALU.mult)
                G1 = p_mid.tile([8, 256], f32, tag="G1")
                nc.vector.scalar_tensor_tensor(
                    G1, TZR[:, 0:256], 1.0, B2, ALU.subtract, ALU.mult)
                hy = p_hy.tile([8, 256], f32, tag="hy")
                nc.vector.tensor_tensor(hy, F1, G1, ALU.subtract)

                psT = p_psT.tile([128, 16], f32, tag="psT")
                for c in range(2):
                    nc.tensor.transpose(psT[:, 8 * c:8 * c + 8],
                                        hy[:, 128 * c:128 * c + 128], ident_s)
                    nc.scalar.copy(
                        hT_s[l][c][:, 8 * (t + 1):8 * (t + 1) + 8],
                        psT[:, 8 * c:8 * c + 8])
                return hy

            for l in range(2):
                emit_bulk(l)
                prev = hy0_s
                for t in range(n_steps):
                    prev = emit_step(l, t, prev)
                for c in range(2):
                    nc.sync.dma_start(hT_d[l, c],
                                      hT_s[l][c][:, 8:8 + 8 * n_steps])
                nc.sync.dma_start(
                    blk_d[l].rearrange("t b k -> b t k"),
                    ky_s[l][:, 4:4 * (n_steps + 1)].rearrange(
                        "b (t k) -> b t k", k=4))

    nc.compile()
    return nc


def _prep_inputs(x, params, n_steps=NSTEP_DEFAULT):
    ST = n_steps
    xT = np.ascontiguousarray(
        x[:ST].reshape(8 * ST, I_SZ).T).reshape(2, 128, 8 * ST)
    wbig = np.zeros((2, 2, 128, 772), np.float32)
    u2t = np.zeros((2, 2, 128, 768), np.float32)
    whkb = np.zeros((2, 2, 128, 4), np.float32)
    rbig = np.zeros((2, 128, 772), np.float32)
    bbhzr = np.zeros((2, 8, 512), np.float32)
    bihn = np.zeros((2, 8, 256), np.float32)
    bhhn2 = np.zeros((2, 8, 256), np.float32)
    for l in range(2):
        w_ik, w_hk, w_ih, w_hh, b_ik, b_ih, b_hh = params[l]
        W2 = w_ih.reshape(G3, I_SZ)
        U2 = w_hh.reshape(G3, H)
        wb = np.concatenate([W2.T, BETA * w_ik], axis=1)  # (256, 772)
        wbig[l] = wb.reshape(2, 128, 772)
        u2t[l] = np.ascontiguousarray(U2.T).reshape(2, 128, 768)
        whkb[l] = (BETA * w_hk).reshape(2, 128, 4)
        rbig[l, :, 768:772] = BETA * b_ik
        bbhzr[l] = np.tile((0.5 * (b_ih + b_hh))[:512], (8, 1))
        bihn[l] = np.tile(b_ih[512:], (8, 1))
        bhhn2[l] = np.tile(0.5 * b_hh[512:], (8, 1))
    kyinit = np.zeros((8, 4), np.float32)
    kyinit[:, 0] = 1.0
    return {
        "xT": np.ascontiguousarray(xT, np.float32),
        "wbig": wbig, "u2t": u2t, "whkb": whkb, "rbig": rbig,
        "bbhzr": bbhzr, "bihn": bihn, "bhhn2": bhhn2,
        "kyinit": kyinit, "ident": np.eye(8, dtype=np.float32),
        "zeros8": np.zeros((128, 8), np.float32),
    }


def _gather_params(inputs):
    return [tuple(np.asarray(inputs[f"{n}{l}"], np.float32) for n in
                  ("w_ik", "w_hk", "w_ih", "w_hh", "b_ik", "b_ih", "b_hh"))
            for l in range(2)]


def kernel(**inputs):
    from concourse.bass_utils import run_bass_kernel_spmd

    x = np.asarray(inputs["x"], np.float32)
    params = []
    for l in range(2):
        w_ik = np.asarray(inputs[f"w_ik{l}"], np.float32)
        w_hk = np.asarray(inputs[f"w_hk{l}"], np.float32)
        w_ih = np.asarray(inputs[f"w_ih{l}"], np.float32)[0]
        w_hh = np.asarray(inputs[f"w_hh{l}"], np.float32)[0]
        b_ik = np.asarray(inputs[f"b_ik{l}"], np.float32)
        b_ih = np.asarray(inputs[f"b_ih{l}"], np.float32)
        b_hh = np.asarray(inputs[f"b_hh{l}"], np.float32)
        params.append((w_ik, w_hk, w_ih, w_hh, b_ik, b_ih, b_hh))

    nc = _build(S, True)
    in_map = _prep_inputs(x, params, S)
    res = run_bass_kernel_spmd(nc, [in_map] * 8, core_ids=list(range(8)))
    out = res.results[0]
    a = np.asarray(out["hT_out"], np.float32)  # (2,2,128,8S)
    hid = np.ascontiguousarray(
        a.reshape(2, 2, 128, S, 8).transpose(0, 3, 4, 1, 2).reshape(2, S, 8, 256))
    blk = np.asarray(out["blk_out"], np.float32)
    cur = hid[1].copy()
    return cur, hid, blk


# revision 12
# speedup vs baseline: 1.1427x; 1.1427x over previous
"""BSGRU (block-switching GRU) Trainium2 Bass kernel.

Math reformulation (validated vs reference to ~1e-6):
  The torch-style .view() on (ky*w_ih) / (kk*w_hh) is equivalent to dense
  matmuls with per-row block scaling:
    i_gates[b,g] = ky[b, g//192] * (x_t @ W2[g]) + b_ih[g],   W2 = w_ih.reshape(768, in)
    h_gates[b,g] = kx[b, g//192]*ky[b, (g%192)//48] * (h @ U2[g]) + b_hh[g]
  sigma(a) = (1+tanh(a/2))/2 so only Exp+Tanh are needed (one ACT table set).

Layout: batch (8) on partitions, gates (768) on free dim.  Per-layer:
  bulk precompute  ibase = X @ W2^T (+ route-x logits) on PE, then a
  64-step scan; state h kept transposed (two (128,8) column blocks per step)
  via per-step PE transposes so the next step's matmuls can contract over h.
All 8 cores run the identical program (scan is latency-bound; extra cores
cannot shorten the sequential critical path) and core 0's output is used.
"""

import functools
import numpy as np

S, B, H, K, BS, I_SZ = 64, 8, 256, 4, 64, 256
G3 = 3 * H  # 768
BETA = 10.0
NSTEP_DEFAULT = S


@functools.lru_cache(maxsize=2)
def _build(n_steps: int = NSTEP_DEFAULT, use_f32r: bool = True):
    import concourse.bass as bass
    import concourse.bacc as bacc
    import concourse.mybir as mybir
    import concourse.tile as tile

    f32 = mybir.dt.float32
    f32r = mybir.dt.float32r
    AF = mybir.ActivationFunctionType
    ALU = mybir.AluOpType

    mmdt = f32r if use_f32r else f32

    nc = bacc.Bacc("TRN2", target_bir_lowering=False, debug=False)

    def din(name, shape, dt=f32):
        return nc.dram_tensor(name, list(shape), dt, kind="ExternalInput").ap()

    ST = n_steps
    xT_d = din("xT", (2, 128, 8 * ST), mmdt)        # x^T h-chunked
    wbig_d = din("wbig", (2, 2, 128, 772), mmdt)    # [W2^T | beta*w_ik] per layer/chunk
    u2t_d = din("u2t", (2, 2, 128, 768), mmdt)
    whkb_d = din("whkb", (2, 2, 128, 4), mmdt)      # beta*w_hk
    rbig_d = din("rbig", (2, 128, 772))       # [0..0 | beta*b_ik] row-replicated
    bbhzr_d = din("bbhzr", (2, 8, 512))       # (b_ih+b_hh)[:512]/2
    bihn_d = din("bihn", (2, 8, 256))         # b_ih[512:]
    bhhn2_d = din("bhhn2", (2, 8, 256))       # b_hh[512:]/2
    kyinit_d = din("kyinit", (8, 4))
    zeros_d = din("zeros8", (128, 8), mmdt)
    ident_d = din("ident", (8, 8))

    hT_d = nc.dram_tensor("hT_out", [2, 2, 128, 8 * ST], mmdt, kind="ExternalOutput").ap()
    blk_d = nc.dram_tensor("blk_out", [2, ST, B, K], f32, kind="ExternalOutput").ap()

    def sb(name, shape, dt=f32):
        return nc.alloc_sbuf_tensor(name, list(shape), dt).ap()

    # persistent SBUF
    xT_s = [sb(f"xT{c}", (128, 8 * ST), mmdt) for c in range(2)]
    wbig_s = [[sb(f"wbig{l}{c}", (128, 772), mmdt) for c in range(2)] for l in range(2)]
    u2t_s = [[sb(f"u2t{l}{c}", (128, 768), mmdt) for c in range(2)] for l in range(2)]
    whkb_s = [[sb(f"whkb{l}{c}", (128, 4), mmdt) for c in range(2)] for l in range(2)]
    rbig_s = [sb(f"rbig{l}", (128, 772)) for l in range(2)]
    bbhzr_s = [sb(f"bbhzr{l}", (8, 512)) for l in range(2)]
    bihn_s = [sb(f"bihn{l}", (8, 256)) for l in range(2)]
    bhhn2_s = [sb(f"bhhn2{l}", (8, 256)) for l in range(2)]
    ident_s = sb("ident_sb", (8, 8))
    bulk_s = [sb(f"bulk{m}", (128, 772)) for m in range((8 * ST + 127) // 128)]
    hT_s = [[sb(f"hT{l}{c}", (128, 8 * (ST + 1)), mmdt) for c in range(2)] for l in range(2)]
    ky_s = [sb(f"kyall{l}", (8, 4 * (ST + 1))) for l in range(2)]
    kyh_s = [sb(f"kyhall{l}", (8, 4 * (ST + 1))) for l in range(2)]
    hy0_s = sb("hy0", (8, 256))

    n_mt = (8 * ST + 127) // 128  # M-tiles in bulk matmul

    with tile.TileContext(nc) as tc:
        with (
            tc.tile_pool(name="ld", bufs=6) as p_ld,
            tc.tile_pool(name="sm", bufs=3) as p_sm,
            tc.tile_pool(name="mid", bufs=3) as p_mid,
            tc.tile_pool(name="hy", bufs=3) as p_hy,
            tc.tile_pool(name="psG", bufs=2, space="PSUM") as p_psG,
            tc.tile_pool(name="psR", bufs=1, space="PSUM") as p_psR,
            tc.tile_pool(name="psT", bufs=1, space="PSUM") as p_psT,
            tc.tile_pool(name="psB", bufs=1, space="PSUM") as p_psB,
        ):
            # ---- init loads ----
            for c in range(2):
                nc.sync.dma_start(xT_s[c], xT_d[c])
            for l in range(2):
                for c in range(2):
                    nc.sync.dma_start(wbig_s[l][c], wbig_d[l, c])
                    nc.sync.dma_start(u2t_s[l][c], u2t_d[l, c])
                    nc.sync.dma_start(whkb_s[l][c], whkb_d[l, c])
                nc.sync.dma_start(rbig_s[l], rbig_d[l])
                nc.sync.dma_start(bbhzr_s[l], bbhzr_d[l])
                nc.sync.dma_start(bihn_s[l], bihn_d[l])
                nc.sync.dma_start(bhhn2_s[l], bhhn2_d[l])
                nc.sync.dma_start(ky_s[l][:, 0:4], kyinit_d)
                nc.scalar.activation(kyh_s[l][:, 0:4], ky_s[l][:, 0:4],
                                     AF.Copy, scale=0.5)
                for c in range(2):
                    nc.sync.dma_start(hT_s[l][c][:, 0:8], zeros_d)
            nc.sync.dma_start(ident_s, ident_d)
            nc.vector.memset(hy0_s, 0.0)

            def emit_bulk(l):
                # ibase/route-x for layer l -> bulk_s tiles (each (128,772))
                for m in range(n_mt):
                    mw = min(128, 8 * ST - 128 * m)
                    psB = p_psB.tile([mw, 772], f32, tag="psB")
                    if l == 0:
                        lhs = [xT_s[c][:, 128 * m:128 * m + mw] for c in range(2)]
                    else:
                        lhs = [hT_s[0][c][:, 8 + 128 * m:8 + 128 * m + mw]
                               for c in range(2)]
                    for lo, hi in ((0, 512), (512, 772)):
                        for c in range(2):
                            nc.tensor.matmul(
                                psB[:, lo:hi], (lhs[c]),
                                (wbig_s[l][c][:, lo:hi]),
                                start=(c == 0), stop=(c == 1))
                    nc.vector.tensor_tensor(bulk_s[m][0:mw, :], psB, rbig_s[l][0:mw, :], ALU.add)

            def emit_step(l, t, prev_hy):
                mt, ro = (8 * t) // 128, (8 * t) % 128
                ibrx = p_ld.tile([8, 772], f32, tag="ibrx")
                nc.sync.dma_start(ibrx, bulk_s[mt][ro:ro + 8, :])

                hT_c = [hT_s[l][c][:, 8 * t:8 * t + 8] for c in range(2)]
                psR = p_psR.tile([8, 4], f32, tag="psR")
                for c in range(2):
                    nc.tensor.matmul(psR, (hT_c[c]), (whkb_s[l][c]),
                                     start=(c == 0), stop=(c == 1))
                psG = p_psG.tile([8, 768], f32, tag="psG")
                for lo, hi in ((0, 512), (512, 768)):
                    for c in range(2):
                        nc.tensor.matmul(psG[:, lo:hi], (hT_c[c]),
                                         (u2t_s[l][c][:, lo:hi]),
                                         start=(c == 0), stop=(c == 1))

                rin = p_sm.tile([8, 4], f32, tag="rin")
                nc.vector.tensor_tensor(rin, psR, ibrx[:, 768:772], ALU.add)
                e_t = p_sm.tile([8, 4], f32, tag="e")
                s_t = p_sm.tile([8, 1], f32, tag="s")
                nc.scalar.activation(e_t, rin, AF.Exp, accum_out=s_t)
                rs = p_sm.tile([8, 1], f32, tag="rs")
                nc.vector.reciprocal(rs, s_t)
                ky_sl = ky_s[l][:, 4 * (t + 1):4 * (t + 1) + 4]
                kx_sl = ky_s[l][:, 4 * t:4 * t + 4]
                nc.vector.tensor_scalar_mul(ky_sl, e_t, rs)
                kyh_sl = kyh_s[l][:, 4 * (t + 1):4 * (t + 1) + 4]
                kxh_sl = kyh_s[l][:, 4 * t:4 * t + 4]
                nc.vector.tensor_scalar_mul(kyh_sl, ky_sl, 0.5)

                kk = p_sm.tile([8, 16], f32, tag="kk")
                kx_r = kx_sl.unsqueeze(2).broadcast_to((8, 4, 4))
                ky_r = ky_sl.unsqueeze(1).broadcast_to((8, 4, 4))
                nc.gpsimd.tensor_tensor(
                    kk.rearrange("p (a b) -> p a b", a=4), kx_r, ky_r, ALU.mult)

                T1 = p_mid.tile([8, 768], f32, tag="T1")
                ky_b = ky_sl.unsqueeze(2).broadcast_to((8, 4, 192))
                nc.vector.tensor_tensor(
                    T1.rearrange("p (a c) -> p a c", a=4), ky_b,
                    ibrx[:, 0:768].rearrange("p (a c) -> p a c", a=4), ALU.mult)
                T2 = p_mid.tile([8, 768], f32, tag="T2")
                kk_b = kk.unsqueeze(2).broadcast_to((8, 16, 48))
                nc.vector.tensor_tensor(
                    T2.rearrange("p (a c) -> p a c", a=16), kk_b,
                    psG.rearrange("p (a c) -> p a c", a=16), ALU.mult)

                Pzr = p_mid.tile([8, 512], f32, tag="Pzr")
                nc.vector.scalar_tensor_tensor(
                    Pzr, T1[:, 0:512], 0.5, bbhzr_s[l], ALU.mult, ALU.add)
                Qzr = p_mid.tile([8, 512], f32, tag="Qzr")
                nc.vector.scalar_tensor_tensor(
                    Qzr, T2[:, 0:512], 0.5, Pzr, ALU.mult, ALU.add)
                TZR = p_mid.tile([8, 512], f32, tag="TZR")
                nc.scalar.activation(TZR[:, 256:512], Qzr[:, 256:512], AF.Tanh)
                nc.scalar.activation(TZR[:, 0:256], Qzr[:, 0:256], AF.Tanh)

                IN = p_mid.tile([8, 256], f32, tag="IN")
                nc.gpsimd.tensor_tensor(IN, T1[:, 512:768], bihn_s[l], ALU.add)
                HN2 = p_mid.tile([8, 256], f32, tag="HN2")
                nc.vector.scalar_tensor_tensor(
                    HN2, T2[:, 512:768], 0.5, bhhn2_s[l], ALU.mult, ALU.add)
                V = p_mid.tile([8, 256], f32, tag="V")
                nc.vector.tensor_tensor(V, HN2, TZR[:, 256:512], ALU.mult)
                W1 = p_mid.tile([8, 256], f32, tag="W1")
                nc.gpsimd.tensor_tensor(W1, IN, HN2, ALU.add)
                narg = p_mid.tile([8, 256], f32, tag="narg")
                nc.vector.tensor_tensor(narg, W1, V, ALU.add)
                TN = p_mid.tile([8, 256], f32, tag="TN")
                nc.scalar.activation(TN, narg, AF.Tanh)

                kxh64 = kxh_sl.unsqueeze(2).broadcast_to((8, 4, 64))
                kyh64 = kyh_sl.unsqueeze(2).broadcast_to((8, 4, 64))
                A2 = p_mid.tile([8, 256], f32, tag="A2")
                nc.gpsimd.tensor_tensor(
                    A2.rearrange("p (a c) -> p a c", a=4),
                    prev_hy.rearrange("p (a c) -> p a c", a=4),
                    kxh64, ALU.mult)
                B2 = p_mid.tile([8, 256], f32, tag="B2")
                nc.vector.tensor_tensor(
                    B2.rearrange("p (a c) -> p a c", a=4),
                    TN.rearrange("p (a c) -> p a c", a=4),
                    kyh64, ALU.mult)
                F1 = p_mid.tile([8, 256], f32, tag="F1")
                nc.vector.scalar_tensor_tensor(
                    F1, TZR[:, 0:256], 1.0, A2, ALU.add, ALU.mult)
                G1 = p_mid.tile([8, 256], f32, tag="G1")
                nc.vector.scalar_tensor_tensor(
                    G1, TZR[:, 0:256], 1.0, B2, ALU.subtract, ALU.mult)
                hy = p_hy.tile([8, 256], f32, tag="hy")
                nc.vector.tensor_tensor(hy, F1, G1, ALU.subtract)

                psT = p_psT.tile([128, 16], f32, tag="psT")
                for c in range(2):
                    nc.tensor.transpose(psT[:, 8 * c:8 * c + 8],
                                        hy[:, 128 * c:128 * c + 128], ident_s)
                    nc.scalar.copy(
                        hT_s[l][c][:, 8 * (t + 1):8 * (t + 1) + 8],
                        psT[:, 8 * c:8 * c + 8])
                return hy

            for l in range(2):
                emit_bulk(l)
                prev = hy0_s
                for t in range(n_steps):
                    prev = emit_step(l, t, prev)
                for c in range(2):
                    nc.sync.dma_start(hT_d[l, c],
                                      hT_s[l][c][:, 8:8 + 8 * n_steps])
                nc.sync.dma_start(
                    blk_d[l].rearrange("t b k -> b t k"),
                    ky_s[l][:, 4:4 * (n_steps + 1)].rearrange(
                        "b (t k) -> b t k", k=4))

    nc.compile()
    return nc


def _prep_inputs(x, params, n_steps=NSTEP_DEFAULT):
    ST = n_steps
    xT = np.ascontiguousarray(
        x[:ST].reshape(8 * ST, I_SZ).T).reshape(2, 128, 8 * ST)
    wbig = np.zeros((2, 2, 128, 772), np.float32)
    u2t = np.zeros((2, 2, 128, 768), np.float32)
    whkb = np.zeros((2, 2, 128, 4), np.float32)
    rbig = np.zeros((2, 128, 772), np.float32)
    bbhzr = np.zeros((2, 8, 512), np.float32)
    bihn = np.zeros((2, 8, 256), np.float32)
    bhhn2 = np.zeros((2, 8, 256), np.float32)
    for l in range(2):
        w_ik, w_hk, w_ih, w_hh, b_ik, b_ih, b_hh = params[l]
        W2 = w_ih.reshape(G3, I_SZ)
        U2 = w_hh.reshape(G3, H)
        wb = np.concatenate([W2.T, BETA * w_ik], axis=1)  # (256, 772)
        wbig[l] = wb.reshape(2, 128, 772)
        u2t[l] = np.ascontiguousarray(U2.T).reshape(2, 128, 768)
        whkb[l] = (BETA * w_hk).reshape(2, 128, 4)
        rbig[l, :, 768:772] = BETA * b_ik
        bbhzr[l] = np.tile((0.5 * (b_ih + b_hh))[:512], (8, 1))
        bihn[l] = np.tile(b_ih[512:], (8, 1))
        bhhn2[l] = np.tile(0.5 * b_hh[512:], (8, 1))
    kyinit = np.zeros((8, 4), np.float32)
    kyinit[:, 0] = 1.0
    return {
        "xT": np.ascontiguousarray(xT, np.float32),
        "wbig": wbig, "u2t": u2t, "whkb": whkb, "rbig": rbig,
        "bbhzr": bbhzr, "bihn": bihn, "bhhn2": bhhn2,
        "kyinit": kyinit, "ident": np.eye(8, dtype=np.float32),
        "zeros8": np.zeros((128, 8), np.float32),
    }


def _gather_params(inputs):
    return [tuple(np.asarray(inputs[f"{n}{l}"], np.float32) for n in
                  ("w_ik", "w_hk", "w_ih", "w_hh", "b_ik", "b_ih", "b_hh"))
            for l in range(2)]


def kernel(**inputs):
    from concourse.bass_utils import run_bass_kernel_spmd

    x = np.asarray(inputs["x"], np.float32)
    params = []
    for l in range(2):
        w_ik = np.asarray(inputs[f"w_ik{l}"], np.float32)
        w_hk = np.asarray(inputs[f"w_hk{l}"], np.float32)
        w_ih = np.asarray(inputs[f"w_ih{l}"], np.float32)[0]
        w_hh = np.asarray(inputs[f"w_hh{l}"], np.float32)[0]
        b_ik = np.asarray(inputs[f"b_ik{l}"], np.float32)
        b_ih = np.asarray(inputs[f"b_ih{l}"], np.float32)
        b_hh = np.asarray(inputs[f"b_hh{l}"], np.float32)
        params.append((w_ik, w_hk, w_ih, w_hh, b_ik, b_ih, b_hh))

    nc = _build(S, True)
    in_map = _prep_inputs(x, params, S)
    res = run_bass_kernel_spmd(nc, [in_map] * 8, core_ids=list(range(8)))
    out = res.results[0]
    a = np.asarray(out["hT_out"], np.float32)  # (2,2,128,8S)
    hid = np.ascontiguousarray(
        a.reshape(2, 2, 128, S, 8).transpose(0, 3, 4, 1, 2).reshape(2, S, 8, 256))
    blk = np.asarray(out["blk_out"], np.float32)
    cur = hid[1].copy()
    return cur, hid, blk
